# revision 42
# baseline (speedup 1.0000x reference)
"""Trainium2 Bass kernel for softmax-free attention:
    q = x @ Wq^T; k = x @ Wk^T; v = x @ Wv^T
    s = (q @ k^T) / sqrt(d); out = s @ v
  x: [4, 4096, 1024], W*: [1024, 1024], out: [4, 4096, 1024] (fp32)

Softmax-free attention is LINEAR, so matmul associativity applies:
    out_b = x_b Wq^T Wk x_b^T x_b Wv^T / sqrt(d)
          = x_b @ M_b,   M_b = Wq_s^T Wk C_b Wv^T,   C_b = x_b^T x_b
with Wq_s = Wq / sqrt(d). This cuts total MACs from 206e9 (explicit
[L,L] score matrix) to ~44e9: per core 256 MMs for C, 256 for the
M-chain, 256 for x@M — 768 N=512 matmuls vs 3600+ in the naive form.

Sharding: 8 cores; core c handles batch c//2, half h = c%2.
  Phase A: core streams its OWN 2048 rows of x_b (natural layout) and
    computes C_own = x_own^T x_own (full [D,D], contraction over its
    rows). C_b = C_own + C_peer. C_own is spilled to cross-core-visible
    Shared DRAM in four column-quarter batches, each followed by its own
    tiny token AllReduce over the pair, so the peer's quarters become
    readable in a pipeline instead of one late barrier.
  Phase B1: U = (C_own + C_peer) @ WvT[:, own 512 cols] accumulated in
    single PSUM groups (own-part MMs run while peer quarters arrive);
    V = Wk @ U; Mc = Wq_s^T @ V = M[:, own cols]. Mc is exchanged with
    the pair partner (spill + token AllReduce) while phase B2 starts on
    the own half. C/U/V/Mc rows are global d indices on all cores; only
    the 512-column j-slice is core-specific (via the pre-sliced WvT
    input), so the SPMD program is identical across cores.
  Phase B2: out[own 2048 rows, :] = x_own @ [Mc_own | Mc_peer], with the
    own column half computed first to hide the Mc exchange. The output
    dram is [2048, 2, 512] with a dynamic slot offset so the local
    own/peer column order maps back to global column halves.

Layout strategy: the PE contracts over the partition dim. C is computed
from x in NATURAL layout (rows on partitions); all later stages need
d-on-partitions operands, which fall out of the previous stage's PSUM
orientation or host-side pre-transposes (xT, Wk^T). C is symmetric, so
C row-chunks serve directly as lhsT tiles without any transpose. The
1/sqrt(d) scale is folded into Wq on the host. All matmul inputs are
float32r (full PE rate at free-dim>=256, ~1e-4 rel err).
"""

import sys
import types
from contextlib import ExitStack

import numpy as np

import concourse.bass as bass
import concourse.tile as tile
from concourse import bacc, mybir
from concourse.bass_utils import run_bass_kernel_spmd
from concourse.mybir import EngineType
from concourse.tile import add_dep_helper
from concourse.vector_clock import ScopedClock

# ---------------------------------------------------------------------------
# Environment shims
# ---------------------------------------------------------------------------


def _install_tile_drain_patch():
    """This toolchain's walrus caps sync waits at 1 per instruction, but
    TileContext's tail drain can carry several. Split the overflow onto
    preceding nops (same semantics: the issuing engine observes every sem
    before draining)."""
    if getattr(tile.TileContext, "_drain_patch_installed", False):
        return

    def _patched_drain_and_barrier(self, tick_clock, wait_clock):
        nc = self.nc
        collector = nc.sync.nop(hint="drain_wait_collector", nofuse=True)
        wait_clock.add_sem_waits(
            collector.ins, ScopedClock({None: tick_clock.global_clock})
        )
        waits = list(collector.ins.sync_info.on_wait or [])
        if len(waits) > 1:
            collector.ins.sync_info.on_wait = [waits[0]]
            for w in waits[1:]:
                nop = nc.sync.nop(hint="drain_wait_extra", nofuse=True)
                nop.ins.sync_info = mybir.SyncInfo(on_wait=[w], on_update=[])
        nc.sync.drain()

        nc.all_engine_barrier()
        assert self.sems is not None
        popped = nc._tile_sem_poison_stack.pop()
        assert popped is self._sem_poison
        nc.clear_and_free_semaphores(list(self.sems.allocated().values()))
        nc.all_engine_barrier()

    tile.TileContext._drain_and_barrier = _patched_drain_and_barrier
    tile.TileContext._drain_patch_installed = True


def _install_ntff_shim():
    """The image's antenv lacks axon_hooks, which silently degrades
    trace=True. Recreate the get/set pair and register the ctypes NTFF hook
    from trn_agent_boot (no-op if unavailable)."""
    if "antenv.axon_hooks" in sys.modules:
        return
    state = {"hook": None}

    def set_axon_ntff_profile_hook(h):
        state["hook"] = h

    def get_axon_ntff_profile_hook():
        return state["hook"]

    mod = types.ModuleType("antenv.axon_hooks")
    mod.set_axon_ntff_profile_hook = set_axon_ntff_profile_hook
    mod.get_axon_ntff_profile_hook = get_axon_ntff_profile_hook
    sys.modules["antenv.axon_hooks"] = mod
    try:
        import antenv

        antenv.axon_hooks = mod
        from trn_agent_boot.trn_boot import _ntff_profile_via_ctypes

        set_axon_ntff_profile_hook(
            _ntff_profile_via_ctypes("/opt/axon/libaxon_pjrt.so")
        )
    except Exception:
        pass


_install_tile_drain_patch()
_install_ntff_shim()

# ---------------------------------------------------------------------------
# Problem constants (hardcoded per the harness contract)
# ---------------------------------------------------------------------------

B, L, D = 4, 4096, 1024
N_CORES = 8
P = 128
LH = L // 2  # rows per core
HD = D // 2  # own output-column half
DC = D // P  # 8 contraction chunks of 128 over d/e
MC = LH // P  # 16 m-chunks of own rows
F32 = mybir.dt.float32
F32R = mybir.dt.float32r
BF16 = mybir.dt.bfloat16

PAIRS = [[2 * i, 2 * i + 1] for i in range(N_CORES // 2)]


def build_nc():
    nc = bacc.Bacc("TRN2", target_bir_lowering=False, debug=False,
                   num_devices=N_CORES)
    xn = nc.dram_tensor("xn", [LH, D], BF16, kind="ExternalInput").ap()
    xth = nc.dram_tensor("xth", [D, LH], F32, kind="ExternalInput").ap()
    wq = nc.dram_tensor("wq", [D, D], F32, kind="ExternalInput").ap()
    wkT = nc.dram_tensor("wkT", [D, D], F32, kind="ExternalInput").ap()
    wvT = nc.dram_tensor("wvT", [D, HD], F32, kind="ExternalInput").ap()
    out = nc.dram_tensor("out", [2, LH, HD], F32,
                         kind="ExternalOutput").ap()
    slots = nc.dram_tensor("slots", [1, 2], mybir.dt.uint32,
                           kind="ExternalInput").ap()
    Csh = nc.dram_tensor("Csh", [2, D, D], F32R, addr_space="Shared").ap()
    Msh = nc.dram_tensor("Msh", [2, D, HD], F32R, addr_space="Shared").ap()
    tokc = [nc.dram_tensor(f"tokc{q}", [1, 2], F32).ap() for q in range(2)]
    tokco = [nc.dram_tensor(f"tokc{q}o", [1, 2], F32).ap() for q in range(2)]
    tokm = nc.dram_tensor("tokm", [1, 2], F32).ap()
    tokmo = nc.dram_tensor("tokmo", [1, 2], F32).ap()
    tokw = nc.dram_tensor("tokw", [1, 2], F32).ap()
    tokwo = nc.dram_tensor("tokwo", [1, 2], F32).ap()
    wu_sink = nc.dram_tensor("wu_sink", [P, 512], F32).ap()

    def chunked(ap):  # [K*, N] dram -> [P, K*/P, N] partition-major
        return ap.rearrange("(c p) n -> p c n", p=P)

    with tile.TileContext(nc) as tc, ExitStack() as octx:
        psum = octx.enter_context(tc.tile_pool(name="psum", bufs=8, space="PSUM"))

        # ---- long-lived left-stack pools -------------------------------
        stage = octx.enter_context(tc.tile_pool(name="stage", bufs=2))

        # zero token source + ncfw warmup AllReduce FIRST: the first
        # collective pays ~30us of firmware cold-start from trigger to
        # mesh-begin; burn it on a dummy immediately so the C-quarter
        # barriers run back-to-back at ~11us each later
        tksrc = stage.tile([1, 2], F32, tag="tks", bufs=1)
        nc.vector.memset(tksrc[:], 0.0)
        nc.sync.dma_start(tokw[:], tksrc[:])
        nc.gpsimd.collective_compute(
            "AllReduce", mybir.AluOpType.add, replica_groups=PAIRS,
            ins=[tokw], outs=[tokwo])

        # rank-in-pair slot selectors for the shared spill buffers
        st_sl = stage.tile([1, 2], mybir.dt.uint32, tag="sl", bufs=1)
        nc.sync.dma_start(st_sl[:], slots[:])
        regs_o = nc.alloc_registers(
            engines=[EngineType.SP, EngineType.Activation,
                     EngineType.Pool])
        nc.regs_load(regs_o, st_sl[0:1, 0:1])
        svo = nc.snap(regs_o, donate=True)
        regs_p = nc.alloc_registers(
            engines=[EngineType.SP, EngineType.Activation,
                     EngineType.Pool])
        nc.regs_load(regs_p, st_sl[0:1, 1:2])
        svp = nc.snap(regs_p, donate=True)

        # HAM warmup: junk matmuls while the first DMAs load, so the PE
        # clock gate is at 8/8 when real work arrives
        with tc.tile_pool(name="wupool", bufs=1) as wupool:
            wut = wupool.tile([P, 512], F32R, tag="wut")
            nc.vector.memset(wut[:].bitcast(F32), 0.0)
            wuo = wupool.tile([P, 512], F32, tag="wuo")
            for g in range(8):
                wp = psum.tile([P, 512], F32, tag="ps", name=f"wu_{g}")
                for r in range(2):
                    nc.tensor.matmul(wp[:], wut[:, 0:P], wut[:],
                                     start=(r == 0), stop=(r == 1))
                if g == 7:
                    nc.vector.tensor_copy(wuo[:], wp[:])
            nc.sync.dma_start(wu_sink[:], wuo[:])

        # wq/wk tiles on the right-side stack (they outlive the cctx pools
        # below but must coexist with phase A's xn stream): wq created
        # before phase A so its load can be issued from the vector engine's
        # stream mid-A
        wqpool = tc.alloc_tile_pool(name="wqpool", bufs=1, side="right")
        octx.callback(wqpool.release)
        wqsb = wqpool.tile([P, DC, D], F32R, tag="wq")
        wkpool = tc.alloc_tile_pool(name="wkpool", bufs=1, side="right")
        octx.callback(wkpool.release)
        wksb = wkpool.tile([P, DC, D], F32R, tag="wk")  # Wk^T, resident

        with ExitStack() as cctx:
            cpool = cctx.enter_context(tc.tile_pool(name="cpool", bufs=1))
            csb = cpool.tile([P, DC, D], F32R, tag="csb")  # C_own rows chunked
            wvpool = cctx.enter_context(tc.tile_pool(name="wvpool", bufs=1))
            wvsb = wvpool.tile([P, DC, HD], F32R, tag="wv")
            cppool = cctx.enter_context(tc.tile_pool(name="cppool", bufs=1))

            # ------------- Phase A: C_own = xn^T xn -----------------------
            # pass 0: cols 0:512 (N=512, paced by the xn stream); passes
            # 1-2: cols 512:768 / 768:1024 (N=256, xn resident) so the
            # second column-half spills ~15us earlier and its
            # AllReduce+peer-read pipeline hides behind the U stage.
            # The token carrying each half's RAW dep is a direct zero
            # write ordered after the spill via an explicit dep (the
            # sample-the-spill dance costs two extra ~2us DMA hops).
            def spill_half(hh):
                cs = slice(hh * HD, (hh + 1) * HD)
                sp = nc.sync.dma_start(
                    Csh[bass.ds(svo, 1), :, cs].rearrange(
                        "s (c p) n -> p (s c) n", p=P),
                    csb[:, :, cs])
                tk = nc.sync.dma_start(tokc[hh][:], tksrc[:])
                add_dep_helper(tk.ins, sp.ins,
                               reason="token after C half spill")

            with ExitStack() as actx:
                xpool = actx.enter_context(tc.tile_pool(name="xpool", bufs=1))
                xnsb = xpool.tile([P, MC, D], BF16, tag="xn")
                # stream own rows (bf16) across the two HWDGE queues.
                # C sums over rows, so any 128-row grouping per chunk is
                # valid — the (p c) layout makes each partition's DMA
                # segment 8KB-contiguous (4 consecutive rows), cutting the
                # descriptor count 4x vs the (c p) layout
                xnpc = xn.rearrange("(p c) n -> p c n", c=MC)
                engs = [nc.scalar, nc.sync]
                xlds = []
                for m4 in range(MC // 4):
                    xlds.append(engs[m4 % 2].dma_start(
                        xnsb[:, 4 * m4:4 * m4 + 4],
                        xnpc[:, 4 * m4:4 * m4 + 4]))

                # weight prefetches: wv (needed first, U stage) and wq
                # (needed last, Mc stage) on the gpsimd/SWDGE ring from
                # t=0; wk (V stage) on scalar behind the xn chunks
                nc.gpsimd.dma_start(wvsb[:], chunked(wvT).bitcast(F32R))
                nc.gpsimd.dma_start(wqsb[:], chunked(wq).bitcast(F32R))
                nc.scalar.dma_start(wksb[:], chunked(wkT).bitcast(F32R))

                passes = [(0, 512), (512, 768), (768, 1024)]
                for pi, (c0, c1) in enumerate(passes):
                    pts = []
                    for d1 in range(DC):
                        pt = psum.tile([P, 512], F32, tag="ps",
                                       name=f"c_{pi}_{d1}")
                        pts.append(pt)
                    w = c1 - c0
                    for m in range(MC):
                        for d1 in range(DC):
                            nc.tensor.matmul(
                                pts[d1][:, 0:w],
                                xnsb[:, m, d1 * P:(d1 + 1) * P],
                                xnsb[:, m, c0:c1],
                                start=(m == 0), stop=(m == MC - 1))
                    for d1 in range(DC):
                        nc.vector.tensor_copy(
                            csb[:, d1, c0:c1], pts[d1][:, 0:w])
                    # spill completed column-halves + their tokens
                    if pi == 0:
                        spill_half(0)
                    elif pi == 2:
                        spill_half(1)

                # pair barriers: half h's AllReduce completes only when
                # BOTH pair members' half-h spills are durable
                cbars = []
                for q in range(2):
                    bar = nc.gpsimd.collective_compute(
                        "AllReduce", mybir.AluOpType.add,
                        replica_groups=PAIRS, ins=[tokc[q]], outs=[tokco[q]])
                    cbars.append(bar)

            # ------------- Phase B1: U = C @ WvT_own ----------------------
            # right-stack pools for the B-phase long-lived tensors
            bpool = tc.alloc_tile_pool(name="bpool", bufs=1, side="right")
            octx.callback(bpool.release)
            usb = bpool.tile([P, DC, HD], F32R, tag="usb")
            vsb = bpool.tile([P, DC, HD], F32R, tag="vsb")
            mcsb = bpool.tile([P, DC, D], F32R, tag="mcsb")

            upts = []
            for d1 in range(DC):
                pt = psum.tile([P, 512], F32, tag="ps", name=f"u_{d1}")
                upts.append(pt)
            # own-part MMs: run immediately, covering the barrier latency
            for d1 in range(DC):
                for c in range(DC):
                    nc.tensor.matmul(
                        upts[d1][:], csb[:, c, d1 * P:(d1 + 1) * P],
                        wvsb[:, c], start=(c == 0), stop=False)
            # peer-part MMs: column-half h of C_peer serves U groups
            # {4h..4h+3}; halves stream in as their barriers complete.
            # After half 0, V groups 0-3 x chunks 0-3 are emitted to keep
            # the PE busy while half 1's exchange pipeline completes
            # (PSUM: 4 open U groups + 4 V groups = 8 banks).
            vpts = [None] * DC
            for hh in range(2):
                cp = cppool.tile([P, DC, HD], F32R, tag="cp",
                                 name=f"cp_{hh}")
                ld = nc.scalar.dma_start(
                    cp[:], Csh[bass.ds(svp, 1), :,
                               hh * HD:(hh + 1) * HD].rearrange(
                        "s (c p) n -> p (s c) n", p=P))
                add_dep_helper(ld.ins, cbars[hh].ins,
                               reason="peer C half after pair barrier")
                for dq in range(4):
                    d1 = hh * 4 + dq
                    for c in range(DC):
                        nc.tensor.matmul(
                            upts[d1][:], cp[:, c, dq * P:(dq + 1) * P],
                            wvsb[:, c], start=False, stop=(c == DC - 1))
                for dq in range(4):
                    d1 = hh * 4 + dq
                    nc.vector.tensor_copy(usb[:, d1], upts[d1][:])
                if hh == 0:
                    for eb in range(4):
                        vpts[eb] = psum.tile([P, 512], F32, tag="ps",
                                             name=f"v_{eb}")
                    for c in range(4):
                        for eb in range(4):
                            nc.tensor.matmul(
                                vpts[eb][:], wksb[:, c, eb * P:(eb + 1) * P],
                                usb[:, c], start=(c == 0), stop=False)

        # csb/wvsb/cpeer released here; xth takes the space. Supertiles
        # split scalar/gpsimd — the SWDGE ring alone caps at ~85GB/s and
        # starved B2 when it carried all 8MB
        xthpool = tc.alloc_tile_pool(name="xthpool", bufs=1, side="right")
        octx.callback(xthpool.release)
        xthsb = xthpool.tile([P, DC, LH], F32R, tag="xth")
        # host permutes Wq's columns so that d-chunk c places d = p*8+c on
        # partition p; partition p's xth DMA segment is then 2 consecutive
        # 8KB rows per slice (the B2 contraction only needs xth and Mc to
        # agree on the d <-> (c,p) mapping)
        xthpc = xth.rearrange("(p c) n -> p c n", c=DC).bitcast(F32R)
        for g in range(4):
            eng = nc.scalar if g < 2 else nc.gpsimd
            eng.dma_start(
                xthsb[:, 2 * g:2 * g + 2, :], xthpc[:, 2 * g:2 * g + 2, :])
        opool = tc.alloc_tile_pool(name="opool", bufs=6, side="right")
        octx.callback(opool.release)

        # ------------- V = Wk @ U tail ------------------------------------
        for c in range(4, DC):
            for eb in range(4):
                nc.tensor.matmul(
                    vpts[eb][:], wksb[:, c, eb * P:(eb + 1) * P],
                    usb[:, c], start=False, stop=(c == DC - 1))
        for eb in range(4):
            nc.vector.tensor_copy(vsb[:, eb], vpts[eb][:])
        for eb in range(4, DC):
            vpts[eb] = psum.tile([P, 512], F32, tag="ps", name=f"v_{eb}")
        for c in range(DC):
            for eb in range(4, DC):
                nc.tensor.matmul(
                    vpts[eb][:], wksb[:, c, eb * P:(eb + 1) * P],
                    usb[:, c], start=(c == 0), stop=(c == DC - 1))
        for eb in range(4, DC):
            nc.vector.tensor_copy(vsb[:, eb], vpts[eb][:])

        # ------------- Mc = Wq_s^T @ V = M[:, own cols] -------------------
        mpts = []
        for ab in range(DC):
            mpts.append(psum.tile([P, 512], F32, tag="ps", name=f"m_{ab}"))
        for c in range(DC):
            for ab in range(DC):
                nc.tensor.matmul(
                    mpts[ab][:], wqsb[:, c, ab * P:(ab + 1) * P],
                    vsb[:, c], start=(c == 0), stop=(c == DC - 1))
        for ab in range(DC):
            nc.vector.tensor_copy(mcsb[:, ab, 0:HD], mpts[ab][:])

        # Mc exchange: spill own columns, token AllReduce, read peer's
        msp = nc.sync.dma_start(
            Msh[bass.ds(svo, 1), :, :].rearrange("s (c p) n -> p (s c) n",
                                                 p=P),
            mcsb[:, :, 0:HD])
        mtk = nc.sync.dma_start(tokm[:], tksrc[:])
        add_dep_helper(mtk.ins, msp.ins, reason="token after Mc spill")
        mbar = nc.gpsimd.collective_compute(
            "AllReduce", mybir.AluOpType.add, replica_groups=PAIRS,
            ins=[tokm], outs=[tokmo])
        # ------------- Phase B2: out = x_own @ [Mc_own | Mc_peer] ---------
        # dh=0 is the own column half (computed first, hiding the exchange);
        # the dynamic slot offset maps it back to the global column half.
        # The peer-Mc read is emitted between the passes so it doesn't
        # block the dh0 out-writes on the scalar ring; out-writes alternate
        # sync/scalar so neither ring backlogs.
        def b2_pass(dh, sl):
            for lb in range(LH // P):
                pt = psum.tile([P, 512], F32, tag="ps", name=f"o_{dh}_{lb}")
                for c in range(DC):
                    nc.tensor.matmul(
                        pt[:], xthsb[:, c, lb * P:(lb + 1) * P],
                        mcsb[:, c, dh * HD:(dh + 1) * HD],
                        start=(c == 0), stop=(c == DC - 1))
                ot = opool.tile([P, 1, HD], F32, tag="ot",
                                name=f"ot_{dh}_{lb}")
                nc.vector.tensor_copy(ot[:, 0], pt[:])
                eng = nc.sync if lb % 2 == 0 else nc.scalar
                eng.dma_start(
                    out[bass.ds(sl, 1), lb * P:(lb + 1) * P, :].rearrange(
                        "s p n -> p s n"), ot[:])

        b2_pass(0, svo)
        mld = nc.scalar.dma_start(
            mcsb[:, :, HD:D],
            Msh[bass.ds(svp, 1), :, :].rearrange("s (c p) n -> p (s c) n",
                                                 p=P))
        add_dep_helper(mld.ins, mbar.ins, reason="peer Mc after pair barrier")
        b2_pass(1, svp)

    nc.compile()
    return nc


_NC_CACHE = {}


def _get_nc():
    if "nc" not in _NC_CACHE:
        _NC_CACHE["nc"] = build_nc()
    return _NC_CACHE["nc"]


def run(inputs, trace=False):
    """Run the kernel on all 8 cores. Returns (full_output, BassKernelResults)."""
    import ml_dtypes

    bf16 = ml_dtypes.bfloat16
    x = np.asarray(inputs["x"], dtype=np.float32)
    Wq = np.asarray(inputs["Wq"], dtype=np.float32)
    Wk = np.asarray(inputs["Wk"], dtype=np.float32)
    Wv = np.asarray(inputs["Wv"], dtype=np.float32)

    inv_sqrt_d = np.float32(1.0 / np.sqrt(D))
    # wq columns permuted so Mc-group c emits rows d = p*8+c on partition
    # p, matching xth's partition-contiguous (p c) layout
    perm = np.arange(D).reshape(P, DC).T.reshape(-1)  # perm[c*128+p]=p*8+c
    wq_s = np.ascontiguousarray((Wq * inv_sqrt_d)[:, perm])
    wkT = np.ascontiguousarray(Wk.T)
    wvT = np.ascontiguousarray(Wv.T)

    in_maps = []
    for c in range(N_CORES):
        b, h = c // 2, c % 2
        in_maps.append({
            "xn": np.ascontiguousarray(
                x[b, h * LH:(h + 1) * LH, :]).astype(bf16),
            "xth": np.ascontiguousarray(
                x[b].T[:, h * LH:(h + 1) * LH]),
            "wq": wq_s, "wkT": wkT,
            "wvT": np.ascontiguousarray(wvT[:, h * HD:(h + 1) * HD]),
            "slots": np.array([[h, 1 - h]], dtype=np.uint32),
        })

    nc = _get_nc()
    res = run_bass_kernel_spmd(nc, in_maps, list(range(N_CORES)), trace=trace)

    full = np.empty((B, L, D), dtype=np.float32)
    for c in range(N_CORES):
        b, h = c // 2, c % 2
        oc = res.results[c]["out"]  # [2, LH, HD]; dim 0 = global col half
        full[b, h * LH:(h + 1) * LH, :] = (
            np.asarray(oc).astype(np.float32).transpose(1, 0, 2)
            .reshape(LH, D))
    return full, res


def kernel(**inputs):
    full, _ = run(inputs, trace=False)
    return full


# revision 43
# speedup vs baseline: 1.0430x; 1.0430x over previous
"""Trainium2 Bass kernel for softmax-free attention:
    q = x @ Wq^T; k = x @ Wk^T; v = x @ Wv^T
    s = (q @ k^T) / sqrt(d); out = s @ v
  x: [4, 4096, 1024], W*: [1024, 1024], out: [4, 4096, 1024] (fp32)

Softmax-free attention is LINEAR, so matmul associativity applies:
    out_b = x_b Wq^T Wk x_b^T x_b Wv^T / sqrt(d)
          = x_b @ M_b,   M_b = Wq_s^T Wk C_b Wv^T,   C_b = x_b^T x_b
with Wq_s = Wq / sqrt(d). This cuts total MACs from 206e9 (explicit
[L,L] score matrix) to ~44e9: per core 256 MMs for C, 256 for the
M-chain, 256 for x@M — 768 N=512 matmuls vs 3600+ in the naive form.

Sharding: 8 cores; core c handles batch c//2, half h = c%2.
  Phase A: core streams its OWN 2048 rows of x_b (natural layout) and
    computes C_own = x_own^T x_own (full [D,D], contraction over its
    rows). C_b = C_own + C_peer. C_own is spilled to cross-core-visible
    Shared DRAM in four column-quarter batches, each followed by its own
    tiny token AllReduce over the pair, so the peer's quarters become
    readable in a pipeline instead of one late barrier.
  Phase B1: U = (C_own + C_peer) @ WvT[:, own 512 cols] accumulated in
    single PSUM groups (own-part MMs run while peer quarters arrive);
    V = Wk @ U; Mc = Wq_s^T @ V = M[:, own cols]. Mc is exchanged with
    the pair partner (spill + token AllReduce) while phase B2 starts on
    the own half. C/U/V/Mc rows are global d indices on all cores; only
    the 512-column j-slice is core-specific (via the pre-sliced WvT
    input), so the SPMD program is identical across cores.
  Phase B2: out[own 2048 rows, :] = x_own @ [Mc_own | Mc_peer], with the
    own column half computed first to hide the Mc exchange. The output
    dram is [2048, 2, 512] with a dynamic slot offset so the local
    own/peer column order maps back to global column halves.

Layout strategy: the PE contracts over the partition dim. C is computed
from x in NATURAL layout (rows on partitions); all later stages need
d-on-partitions operands, which fall out of the previous stage's PSUM
orientation or host-side pre-transposes (xT, Wk^T). C is symmetric, so
C row-chunks serve directly as lhsT tiles without any transpose. The
1/sqrt(d) scale is folded into Wq on the host. All matmul inputs are
float32r (full PE rate at free-dim>=256, ~1e-4 rel err).
"""

import sys
import types
from contextlib import ExitStack

import numpy as np

import concourse.bass as bass
import concourse.tile as tile
from concourse import bacc, mybir
from concourse.bass_utils import run_bass_kernel_spmd
from concourse.mybir import EngineType
from concourse.tile import add_dep_helper
from concourse.vector_clock import ScopedClock

# ---------------------------------------------------------------------------
# Environment shims
# ---------------------------------------------------------------------------


def _install_tile_drain_patch():
    """This toolchain's walrus caps sync waits at 1 per instruction, but
    TileContext's tail drain can carry several. Split the overflow onto
    preceding nops (same semantics: the issuing engine observes every sem
    before draining)."""
    if getattr(tile.TileContext, "_drain_patch_installed", False):
        return

    def _patched_drain_and_barrier(self, tick_clock, wait_clock):
        nc = self.nc
        collector = nc.sync.nop(hint="drain_wait_collector", nofuse=True)
        wait_clock.add_sem_waits(
            collector.ins, ScopedClock({None: tick_clock.global_clock})
        )
        waits = list(collector.ins.sync_info.on_wait or [])
        if len(waits) > 1:
            collector.ins.sync_info.on_wait = [waits[0]]
            for w in waits[1:]:
                nop = nc.sync.nop(hint="drain_wait_extra", nofuse=True)
                nop.ins.sync_info = mybir.SyncInfo(on_wait=[w], on_update=[])
        nc.sync.drain()

        nc.all_engine_barrier()
        assert self.sems is not None
        popped = nc._tile_sem_poison_stack.pop()
        assert popped is self._sem_poison
        nc.clear_and_free_semaphores(list(self.sems.allocated().values()))
        nc.all_engine_barrier()

    tile.TileContext._drain_and_barrier = _patched_drain_and_barrier
    tile.TileContext._drain_patch_installed = True


def _install_ntff_shim():
    """The image's antenv lacks axon_hooks, which silently degrades
    trace=True. Recreate the get/set pair and register the ctypes NTFF hook
    from trn_agent_boot (no-op if unavailable)."""
    if "antenv.axon_hooks" in sys.modules:
        return
    state = {"hook": None}

    def set_axon_ntff_profile_hook(h):
        state["hook"] = h

    def get_axon_ntff_profile_hook():
        return state["hook"]

    mod = types.ModuleType("antenv.axon_hooks")
    mod.set_axon_ntff_profile_hook = set_axon_ntff_profile_hook
    mod.get_axon_ntff_profile_hook = get_axon_ntff_profile_hook
    sys.modules["antenv.axon_hooks"] = mod
    try:
        import antenv

        antenv.axon_hooks = mod
        from trn_agent_boot.trn_boot import _ntff_profile_via_ctypes

        set_axon_ntff_profile_hook(
            _ntff_profile_via_ctypes("/opt/axon/libaxon_pjrt.so")
        )
    except Exception:
        pass


_install_tile_drain_patch()
_install_ntff_shim()

# ---------------------------------------------------------------------------
# Problem constants (hardcoded per the harness contract)
# ---------------------------------------------------------------------------

B, L, D = 4, 4096, 1024
N_CORES = 8
P = 128
LH = L // 2  # rows per core
HD = D // 2  # own output-column half
DC = D // P  # 8 contraction chunks of 128 over d/e
MC = LH // P  # 16 m-chunks of own rows
F32 = mybir.dt.float32
F32R = mybir.dt.float32r
BF16 = mybir.dt.bfloat16

PAIRS = [[2 * i, 2 * i + 1] for i in range(N_CORES // 2)]


def build_nc():
    nc = bacc.Bacc("TRN2", target_bir_lowering=False, debug=False,
                   num_devices=N_CORES)
    xn = nc.dram_tensor("xn", [LH, D], BF16, kind="ExternalInput").ap()
    xth = nc.dram_tensor("xth", [D, LH], F32, kind="ExternalInput").ap()
    wq = nc.dram_tensor("wq", [D, D], F32, kind="ExternalInput").ap()
    wkT = nc.dram_tensor("wkT", [D, D], F32, kind="ExternalInput").ap()
    wvT = nc.dram_tensor("wvT", [D, HD], F32, kind="ExternalInput").ap()
    out = nc.dram_tensor("out", [2, LH, HD], F32,
                         kind="ExternalOutput").ap()
    slots = nc.dram_tensor("slots", [1, 2], mybir.dt.uint32,
                           kind="ExternalInput").ap()
    Csh = nc.dram_tensor("Csh", [2, D, D], F32R, addr_space="Shared").ap()
    Msh = nc.dram_tensor("Msh", [2, D, HD], F32R, addr_space="Shared").ap()
    tokc = [nc.dram_tensor(f"tokc{q}", [1, 2], F32).ap() for q in range(2)]
    tokco = [nc.dram_tensor(f"tokc{q}o", [1, 2], F32).ap() for q in range(2)]
    tokm = nc.dram_tensor("tokm", [1, 2], F32).ap()
    tokmo = nc.dram_tensor("tokmo", [1, 2], F32).ap()
    tokw = nc.dram_tensor("tokw", [1, 2], F32).ap()
    tokwo = nc.dram_tensor("tokwo", [1, 2], F32).ap()
    wu_sink = nc.dram_tensor("wu_sink", [P, 512], F32).ap()

    def chunked(ap):  # [K*, N] dram -> [P, K*/P, N] partition-major
        return ap.rearrange("(c p) n -> p c n", p=P)

    with tile.TileContext(nc) as tc, ExitStack() as octx:
        psum = octx.enter_context(tc.tile_pool(name="psum", bufs=8, space="PSUM"))

        # ---- long-lived left-stack pools -------------------------------
        stage = octx.enter_context(tc.tile_pool(name="stage", bufs=2))

        # zero token source + ncfw warmup AllReduce FIRST: the first
        # collective pays ~30us of firmware cold-start from trigger to
        # mesh-begin; burn it on a dummy immediately so the C-quarter
        # barriers run back-to-back at ~11us each later
        tksrc = stage.tile([1, 2], F32, tag="tks", bufs=1)
        nc.vector.memset(tksrc[:], 0.0)
        nc.sync.dma_start(tokw[:], tksrc[:])
        nc.gpsimd.collective_compute(
            "AllReduce", mybir.AluOpType.add, replica_groups=PAIRS,
            ins=[tokw], outs=[tokwo])

        # rank-in-pair slot selectors for the shared spill buffers
        st_sl = stage.tile([1, 2], mybir.dt.uint32, tag="sl", bufs=1)
        nc.sync.dma_start(st_sl[:], slots[:])
        regs_o = nc.alloc_registers(
            engines=[EngineType.SP, EngineType.Activation,
                     EngineType.Pool])
        nc.regs_load(regs_o, st_sl[0:1, 0:1])
        svo = nc.snap(regs_o, donate=True)
        regs_p = nc.alloc_registers(
            engines=[EngineType.SP, EngineType.Activation,
                     EngineType.Pool])
        nc.regs_load(regs_p, st_sl[0:1, 1:2])
        svp = nc.snap(regs_p, donate=True)

        # HAM warmup: junk matmuls while the first DMAs load, so the PE
        # clock gate is at 8/8 when real work arrives
        with tc.tile_pool(name="wupool", bufs=1) as wupool:
            wut = wupool.tile([P, 512], F32R, tag="wut")
            nc.vector.memset(wut[:].bitcast(F32), 0.0)
            wuo = wupool.tile([P, 512], F32, tag="wuo")
            for g in range(8):
                wp = psum.tile([P, 512], F32, tag="ps", name=f"wu_{g}")
                for r in range(2):
                    nc.tensor.matmul(wp[:], wut[:, 0:P], wut[:],
                                     start=(r == 0), stop=(r == 1))
                if g == 7:
                    nc.vector.tensor_copy(wuo[:], wp[:])
            nc.sync.dma_start(wu_sink[:], wuo[:])

        # wq/wk tiles on the right-side stack (they outlive the cctx pools
        # below but must coexist with phase A's xn stream): wq created
        # before phase A so its load can be issued from the vector engine's
        # stream mid-A
        wqpool = tc.alloc_tile_pool(name="wqpool", bufs=1, side="right")
        octx.callback(wqpool.release)
        wqsb = wqpool.tile([P, DC, D], F32R, tag="wq")
        wkpool = tc.alloc_tile_pool(name="wkpool", bufs=1, side="right")
        octx.callback(wkpool.release)
        wksb = wkpool.tile([P, DC, D], F32R, tag="wk")  # Wk^T, resident

        with ExitStack() as cctx:
            cpool = cctx.enter_context(tc.tile_pool(name="cpool", bufs=1))
            csb = cpool.tile([P, DC, D], F32R, tag="csb")  # C_own rows chunked
            wvpool = cctx.enter_context(tc.tile_pool(name="wvpool", bufs=1))
            wvsb = wvpool.tile([P, DC, HD], F32R, tag="wv")
            cppool = cctx.enter_context(tc.tile_pool(name="cppool", bufs=1))

            # ------------- Phase A: C_own = xn^T xn -----------------------
            # pass 0: cols 0:512 (N=512, paced by the xn stream); passes
            # 1-2: cols 512:768 / 768:1024 (N=256, xn resident) so the
            # second column-half spills ~15us earlier and its
            # AllReduce+peer-read pipeline hides behind the U stage.
            # The token carrying each half's RAW dep is a direct zero
            # write ordered after the spill via an explicit dep (the
            # sample-the-spill dance costs two extra ~2us DMA hops).
            def spill_half(hh):
                cs = slice(hh * HD, (hh + 1) * HD)
                sp = nc.sync.dma_start(
                    Csh[bass.ds(svo, 1), :, cs].rearrange(
                        "s (c p) n -> p (s c) n", p=P),
                    csb[:, :, cs])
                tk = nc.sync.dma_start(tokc[hh][:], tksrc[:])
                add_dep_helper(tk.ins, sp.ins,
                               reason="token after C half spill")

            with ExitStack() as actx:
                xpool = actx.enter_context(tc.tile_pool(name="xpool", bufs=1))
                xnsb = xpool.tile([P, MC, D], BF16, tag="xn")
                # stream own rows (bf16) across the two HWDGE queues.
                # C sums over rows, so any 128-row grouping per chunk is
                # valid — the (p c) layout makes each partition's DMA
                # segment 8KB-contiguous (4 consecutive rows), cutting the
                # descriptor count 4x vs the (c p) layout
                xnpc = xn.rearrange("(p c) n -> p c n", c=MC)
                engs = [nc.scalar, nc.sync]
                xlds = []
                for m4 in range(MC // 4):
                    xlds.append(engs[m4 % 2].dma_start(
                        xnsb[:, 4 * m4:4 * m4 + 4],
                        xnpc[:, 4 * m4:4 * m4 + 4]))

                # weight prefetches: wv (needed first, U stage) and wq
                # (needed last, Mc stage) on the gpsimd/SWDGE ring from
                # t=0; wk (V stage) on scalar behind the xn chunks
                nc.gpsimd.dma_start(wvsb[:], chunked(wvT).bitcast(F32R))
                nc.gpsimd.dma_start(wksb[:], chunked(wkT).bitcast(F32R))
                nc.gpsimd.dma_start(wqsb[:], chunked(wq).bitcast(F32R))

                passes = [(0, 512), (512, 768), (768, 1024)]
                for pi, (c0, c1) in enumerate(passes):
                    pts = []
                    for d1 in range(DC):
                        pt = psum.tile([P, 512], F32, tag="ps",
                                       name=f"c_{pi}_{d1}")
                        pts.append(pt)
                    w = c1 - c0
                    for m in range(MC):
                        for d1 in range(DC):
                            nc.tensor.matmul(
                                pts[d1][:, 0:w],
                                xnsb[:, m, d1 * P:(d1 + 1) * P],
                                xnsb[:, m, c0:c1],
                                start=(m == 0), stop=(m == MC - 1))
                    for d1 in range(DC):
                        nc.vector.tensor_copy(
                            csb[:, d1, c0:c1], pts[d1][:, 0:w])
                    # spill completed column-halves + their tokens
                    if pi == 0:
                        spill_half(0)
                    elif pi == 2:
                        spill_half(1)

                # pair barriers: half h's AllReduce completes only when
                # BOTH pair members' half-h spills are durable
                cbars = []
                for q in range(2):
                    bar = nc.gpsimd.collective_compute(
                        "AllReduce", mybir.AluOpType.add,
                        replica_groups=PAIRS, ins=[tokc[q]], outs=[tokco[q]])
                    cbars.append(bar)

            # ------------- Phase B1: U = C @ WvT_own ----------------------
            # right-stack pools for the B-phase long-lived tensors
            bpool = tc.alloc_tile_pool(name="bpool", bufs=1, side="right")
            octx.callback(bpool.release)
            usb = bpool.tile([P, DC, HD], F32R, tag="usb")
            vsb = bpool.tile([P, DC, HD], F32R, tag="vsb")
            mcsb = bpool.tile([P, DC, D], F32R, tag="mcsb")

            upts = []
            for d1 in range(DC):
                pt = psum.tile([P, 512], F32, tag="ps", name=f"u_{d1}")
                upts.append(pt)
            # own-part MMs: run immediately, covering the barrier latency
            for d1 in range(DC):
                for c in range(DC):
                    nc.tensor.matmul(
                        upts[d1][:], csb[:, c, d1 * P:(d1 + 1) * P],
                        wvsb[:, c], start=(c == 0), stop=False)
            # peer-part MMs: column-half h of C_peer serves U groups
            # {4h..4h+3}; halves stream in as their barriers complete.
            # After half 0, V groups 0-3 x chunks 0-3 are emitted to keep
            # the PE busy while half 1's exchange pipeline completes
            # (PSUM: 4 open U groups + 4 V groups = 8 banks).
            vpts = [None] * DC
            for hh in range(2):
                cp = cppool.tile([P, DC, HD], F32R, tag="cp",
                                 name=f"cp_{hh}")
                ld = nc.scalar.dma_start(
                    cp[:], Csh[bass.ds(svp, 1), :,
                               hh * HD:(hh + 1) * HD].rearrange(
                        "s (c p) n -> p (s c) n", p=P))
                add_dep_helper(ld.ins, cbars[hh].ins,
                               reason="peer C half after pair barrier")
                for dq in range(4):
                    d1 = hh * 4 + dq
                    for c in range(DC):
                        nc.tensor.matmul(
                            upts[d1][:], cp[:, c, dq * P:(dq + 1) * P],
                            wvsb[:, c], start=False, stop=(c == DC - 1))
                for dq in range(4):
                    d1 = hh * 4 + dq
                    nc.vector.tensor_copy(usb[:, d1], upts[d1][:])
                if hh == 0:
                    for eb in range(4):
                        vpts[eb] = psum.tile([P, 512], F32, tag="ps",
                                             name=f"v_{eb}")
                    for c in range(4):
                        for eb in range(4):
                            nc.tensor.matmul(
                                vpts[eb][:], wksb[:, c, eb * P:(eb + 1) * P],
                                usb[:, c], start=(c == 0), stop=False)

        # csb/wvsb/cpeer released here; xth takes the space. Supertiles
        # split scalar/gpsimd — the SWDGE ring alone caps at ~85GB/s and
        # starved B2 when it carried all 8MB
        xthpool = tc.alloc_tile_pool(name="xthpool", bufs=1, side="right")
        octx.callback(xthpool.release)
        xthsb = xthpool.tile([P, DC, LH], F32R, tag="xth")
        # host permutes Wq's columns so that d-chunk c places d = p*8+c on
        # partition p; partition p's xth DMA segment is then 2 consecutive
        # 8KB rows per slice (the B2 contraction only needs xth and Mc to
        # agree on the d <-> (c,p) mapping)
        xthpc = xth.rearrange("(p c) n -> p c n", c=DC).bitcast(F32R)
        for g in range(4):
            eng = nc.scalar if g < 2 else nc.gpsimd
            eng.dma_start(
                xthsb[:, 2 * g:2 * g + 2, :], xthpc[:, 2 * g:2 * g + 2, :])
        opool = tc.alloc_tile_pool(name="opool", bufs=6, side="right")
        octx.callback(opool.release)

        # ------------- V = Wk @ U tail ------------------------------------
        for c in range(4, DC):
            for eb in range(4):
                nc.tensor.matmul(
                    vpts[eb][:], wksb[:, c, eb * P:(eb + 1) * P],
                    usb[:, c], start=False, stop=(c == DC - 1))
        for eb in range(4):
            nc.vector.tensor_copy(vsb[:, eb], vpts[eb][:])
        for eb in range(4, DC):
            vpts[eb] = psum.tile([P, 512], F32, tag="ps", name=f"v_{eb}")
        for c in range(DC):
            for eb in range(4, DC):
                nc.tensor.matmul(
                    vpts[eb][:], wksb[:, c, eb * P:(eb + 1) * P],
                    usb[:, c], start=(c == 0), stop=(c == DC - 1))
        for eb in range(4, DC):
            nc.vector.tensor_copy(vsb[:, eb], vpts[eb][:])

        # ------------- Mc = Wq_s^T @ V = M[:, own cols] -------------------
        mpts = []
        for ab in range(DC):
            mpts.append(psum.tile([P, 512], F32, tag="ps", name=f"m_{ab}"))
        for c in range(DC):
            for ab in range(DC):
                nc.tensor.matmul(
                    mpts[ab][:], wqsb[:, c, ab * P:(ab + 1) * P],
                    vsb[:, c], start=(c == 0), stop=(c == DC - 1))
        for ab in range(DC):
            nc.vector.tensor_copy(mcsb[:, ab, 0:HD], mpts[ab][:])

        # Mc exchange: spill own columns, token AllReduce, read peer's
        msp = nc.sync.dma_start(
            Msh[bass.ds(svo, 1), :, :].rearrange("s (c p) n -> p (s c) n",
                                                 p=P),
            mcsb[:, :, 0:HD])
        mtk = nc.sync.dma_start(tokm[:], tksrc[:])
        add_dep_helper(mtk.ins, msp.ins, reason="token after Mc spill")
        mbar = nc.gpsimd.collective_compute(
            "AllReduce", mybir.AluOpType.add, replica_groups=PAIRS,
            ins=[tokm], outs=[tokmo])
        # ------------- Phase B2: out = x_own @ [Mc_own | Mc_peer] ---------
        # dh=0 is the own column half (computed first, hiding the exchange);
        # the dynamic slot offset maps it back to the global column half.
        # The peer-Mc read is emitted between the passes so it doesn't
        # block the dh0 out-writes on the scalar ring; out-writes alternate
        # sync/scalar so neither ring backlogs.
        def b2_pass(dh, sl):
            for lb in range(LH // P):
                pt = psum.tile([P, 512], F32, tag="ps", name=f"o_{dh}_{lb}")
                for c in range(DC):
                    nc.tensor.matmul(
                        pt[:], xthsb[:, c, lb * P:(lb + 1) * P],
                        mcsb[:, c, dh * HD:(dh + 1) * HD],
                        start=(c == 0), stop=(c == DC - 1))
                ot = opool.tile([P, 1, HD], F32, tag="ot",
                                name=f"ot_{dh}_{lb}")
                nc.vector.tensor_copy(ot[:, 0], pt[:])
                eng = nc.sync if lb % 2 == 0 else nc.scalar
                eng.dma_start(
                    out[bass.ds(sl, 1), lb * P:(lb + 1) * P, :].rearrange(
                        "s p n -> p s n"), ot[:])

        mld = nc.scalar.dma_start(
            mcsb[:, :, HD:D],
            Msh[bass.ds(svp, 1), :, :].rearrange("s (c p) n -> p (s c) n",
                                                 p=P))
        add_dep_helper(mld.ins, mbar.ins, reason="peer Mc after pair barrier")
        b2_pass(0, svo)
        b2_pass(1, svp)

    nc.compile()
    return nc


_NC_CACHE = {}


def _get_nc():
    if "nc" not in _NC_CACHE:
        _NC_CACHE["nc"] = build_nc()
    return _NC_CACHE["nc"]


def run(inputs, trace=False):
    """Run the kernel on all 8 cores. Returns (full_output, BassKernelResults)."""
    import ml_dtypes

    bf16 = ml_dtypes.bfloat16
    x = np.asarray(inputs["x"], dtype=np.float32)
    Wq = np.asarray(inputs["Wq"], dtype=np.float32)
    Wk = np.asarray(inputs["Wk"], dtype=np.float32)
    Wv = np.asarray(inputs["Wv"], dtype=np.float32)

    inv_sqrt_d = np.float32(1.0 / np.sqrt(D))
    # wq columns permuted so Mc-group c emits rows d = p*8+c on partition
    # p, matching xth's partition-contiguous (p c) layout
    perm = np.arange(D).reshape(P, DC).T.reshape(-1)  # perm[c*128+p]=p*8+c
    wq_s = np.ascontiguousarray((Wq * inv_sqrt_d)[:, perm])
    wkT = np.ascontiguousarray(Wk.T)
    wvT = np.ascontiguousarray(Wv.T)

    in_maps = []
    for c in range(N_CORES):
        b, h = c // 2, c % 2
        in_maps.append({
            "xn": np.ascontiguousarray(
                x[b, h * LH:(h + 1) * LH, :]).astype(bf16),
            "xth": np.ascontiguousarray(
                x[b].T[:, h * LH:(h + 1) * LH]),
            "wq": wq_s, "wkT": wkT,
            "wvT": np.ascontiguousarray(wvT[:, h * HD:(h + 1) * HD]),
            "slots": np.array([[h, 1 - h]], dtype=np.uint32),
        })

    nc = _get_nc()
    res = run_bass_kernel_spmd(nc, in_maps, list(range(N_CORES)), trace=trace)

    full = np.empty((B, L, D), dtype=np.float32)
    for c in range(N_CORES):
        b, h = c // 2, c % 2
        oc = res.results[c]["out"]  # [2, LH, HD]; dim 0 = global col half
        full[b, h * LH:(h + 1) * LH, :] = (
            np.asarray(oc).astype(np.float32).transpose(1, 0, 2)
            .reshape(LH, D))
    return full, res


def kernel(**inputs):
    full, _ = run(inputs, trace=False)
    return full
